# revision 9
# baseline (speedup 1.0000x reference)
"""Trainium2 Bass kernel for nn_HarmonicEstimation (topk_masking).

Problem: x [16,1,1025,1024] f32 -> mask [16,1,1025,1024].
Per (batch, t) column over f-bins 1..1024: find top-5 peaks, f0 = min index
among peaks with value > 0.1 (else 0); output column = harmonic-comb mask
that depends ONLY on f0.

Strategy (8 cores, 2 batches/core, no communication):
  - Mask column is a pure function of f0 -> host-precomputed LUT, stored
    bf16 (tolerance 2e-2 >> bf16 error) with rows padded to 1152 elements
    (2304B, 256B-multiple as dma_gather requires). LUT rows are indexed by
    i = f0-1 (i in 0..1023); the invalid-column sentinel i=1023 coincides
    with f0=1024 whose row is all-0.5, exactly the wanted fallback, so the
    raw find_index8 output is directly the gather index.
  - Input loads with ONE DMA per batch (3-dim AP) into a [128, 8*1024]
    natural-layout tile; PE transposes 128x128 blocks into 2-bank
    [128,1024] PSUM tiles.
  - DVE max8/find_index8 read the full column from PSUM in one pass each
    (both have ~0.5us fixed cost - fewer, wider ops win). All small f0
    arithmetic runs on the Pool engine to keep DVE at exactly 2 ops/tile.
    f0 derives from a threshold rule exactly equivalent to top-5
    min-index: f0-1 = min{i : x[i+1] >= max(v5, nextafter(0.1))}.
  - The f0->gather index chain (fold to 16 partitions, replicate to 128
    as the SWDGE ucode requires, descriptor gen) is run per HALF-batch so
    all but the final half's chain hides under phase-1 compute.
  - One dma_gather per (batch, t-half) in TRANSPOSE mode (16-bit): the
    DMA engines gather LUT rows AND transpose them, so dst[p, c, t] =
    LUT[f0(t)-1, 128c+p] - natural [f-part, t-free] bf16 output tiles.
    (num_idxs=1024 in one gather fails on HW; 512 is safe.)
  - Output: ONE DMA per (batch, half) writes all 8 f-chunks (3-dim AP,
    1KB descriptors) + a tiny DMA for the k=1024 row; bf16, upcast on
    host.
"""

import os
import sys

for _p in ("/opt/trn_rl_repo", "/root/.axon_site/_ro/trn_rl_repo"):
    if os.path.isdir(_p) and _p not in sys.path:
        sys.path.insert(0, _p)

import numpy as np
import ml_dtypes

import concourse.bacc as bacc
import concourse.mybir as mybir
from concourse.bass_utils import run_bass_kernel_spmd
from concourse.tile import TileContext

dt = mybir.dt
Alu = mybir.AluOpType

B = 16          # full batch
NB = 2          # batches per core
NCORES = 8
F = 1025        # freq bins (0..1024)
T = 1024        # time columns
FT = 8          # f tiles of 128 covering bins 1..1024
TT = 8          # t tiles of 128
LUT_W = 1152    # LUT row padded to 1152 bf16 = 2304 B (multiple of 256)
NI = 512        # idxs per gather (1024 exceeds a HW ucode limit)
NQ = 4          # SWDGE queues (ucode max)
MAX_POWER = 0.1
# val > 0.1f  <=>  val >= nextafter(0.1f)
MP_NEXT = float(np.nextafter(np.float32(MAX_POWER), np.float32(1)))

_CACHE = {}


def _build_lut() -> np.ndarray:
    """LUT[i, k] = reference mask value at bin k given fundamental f0=i+1,
    bf16, rows padded to LUT_W. Mirrors reference.py arithmetic in f32.
    Rows i >= 1019 (f0 > 1020) are all-0.5; i=1023 doubles as the
    invalid-column sentinel."""
    if "lut" in _CACHE:
        return _CACHE["lut"]
    k = np.arange(F, dtype=np.int64)[None, :]           # [1, k]
    f0 = np.arange(1, 1025, dtype=np.int64)[:, None]    # [i, 1], f0 = i+1
    limit = F - 3 - 2  # F - FREQ_MARGIN - 2 = 1020
    m_mult = np.minimum((k + 3) // f0, limit // f0)
    i_last = m_mult * f0
    dist = np.abs(k - i_last).astype(np.float32)
    val = np.maximum(
        np.float32(1.0) - (np.float32(0.5) * dist) / np.float32(3.0),
        np.float32(0.5),
    )
    ok = (i_last >= f0) & (i_last >= k - 3)
    lut = np.where(ok, val, np.float32(0.5)).astype(np.float32)  # [1024, F]
    out = np.full((1024, LUT_W), 0.5, dtype=ml_dtypes.bfloat16)
    out[:, :F] = lut.astype(ml_dtypes.bfloat16)
    _CACHE["lut"] = out
    return out


def _build_nc():
    if "nc" in _CACHE:
        return _CACHE["nc"]
    from contextlib import ExitStack

    nc = bacc.Bacc(
        "TRN2", target_bir_lowering=False, debug=False, num_swdge_queues=NQ
    )
    x_in = nc.dram_tensor("x", [NB, F, T], dt.float32, kind="ExternalInput").ap()
    lut_d = nc.dram_tensor("lut", [1024, LUT_W], dt.bfloat16, kind="ExternalInput").ap()
    ident_d = nc.dram_tensor("ident", [128, 128], dt.float32, kind="ExternalInput").ap()
    out_d = nc.dram_tensor("out", [NB, F, T], dt.bfloat16, kind="ExternalOutput").ap()

    with TileContext(nc) as tc, ExitStack() as ctx:
        const_pool = ctx.enter_context(tc.tile_pool(name="constp", bufs=1))
        nat_pool = ctx.enter_context(tc.tile_pool(name="natp", bufs=2))
        gg_pool = ctx.enter_context(tc.tile_pool(name="ggp", bufs=4))
        psum_pool = ctx.enter_context(tc.tile_pool(name="psump", bufs=3, space="PSUM"))
        small_pool = ctx.enter_context(tc.tile_pool(name="smallp", bufs=2))

        ident_sb = const_pool.tile([128, 128], dt.float32, name="ident_sb")
        nc.sync.dma_start(ident_sb[:], ident_d[:])

        for b in range(NB):
            # ---- one-DMA load of all 8 natural [f-part, t-free] tiles ----
            # nat[p, h*1024 + t] = x[b, 1 + h*128 + p, t]
            nat = nat_pool.tile([128, FT * T], dt.float32, name=f"nat{b}", tag="nat")
            nc.sync.dma_start(
                nat[:].rearrange("p (h t) -> p h t", t=T),
                x_in[b, 1:, :].rearrange("(h p) t -> p h t", p=128),
            )

            # ---- per t-tile: transpose to PSUM, top-8 (DVE), thr/mask (Pool) ----
            vals = small_pool.tile([128, 8 * TT], dt.float32, name=f"vals{b}", tag="vals")
            idxs = small_pool.tile([128, 8 * TT], dt.uint16, name=f"idxs{b}", tag="idxs")
            mask = small_pool.tile([128, 8 * TT], dt.uint16, name=f"mask{b}", tag="mask")
            thr = small_pool.tile([128, TT], dt.float32, name=f"thr{b}", tag="thr")
            for g in range(TT):
                ps = psum_pool.tile([128, 1024], dt.float32, name=f"ps{b}_{g}", tag="ps")
                for h in range(FT):
                    nc.tensor.transpose(
                        ps[:, h * 128:(h + 1) * 128],
                        nat[:, h * T + g * 128: h * T + (g + 1) * 128],
                        ident_sb[:],
                    )
                vsl = vals[:, 8 * g:8 * g + 8]
                nc.vector.max(vsl, ps[:])
                nc.vector.max_index(idxs[:, 8 * g:8 * g + 8], vsl, ps[:])
                nc.gpsimd.tensor_scalar(
                    thr[:, g:g + 1], vals[:, 8 * g + 4:8 * g + 5],
                    MP_NEXT, None, Alu.max,
                )
                nc.gpsimd.tensor_scalar(
                    mask[:, 8 * g:8 * g + 8], vsl,
                    thr[:, g:g + 1], None, Alu.is_ge,
                )

            # ---- per t-half: f0-1, fold, replicate, gather, write out ----
            wrapped = small_pool.tile([128, 8 * TT], dt.int16, name=f"wrap{b}", tag="wrap")
            wv = wrapped[:].rearrange("p (g a) -> p g a", a=8)
            for j in range(2):
                sl = slice(32 * j, 32 * (j + 1))        # slots of this half
                # cand = mask ? idx : 1023 ; f0h = min over slots
                cand = small_pool.tile([128, 32], dt.uint16, name=f"cand{b}{j}", tag="cand")
                f0h = small_pool.tile([128, 4], dt.int16, name=f"f0h{b}{j}", tag="f0h")
                nc.vector.memset(cand[:], 1023)
                nc.vector.copy_predicated(cand[:], mask[:, sl], idxs[:, sl])
                nc.vector.tensor_reduce(
                    f0h[:], cand[:].rearrange("p (g s) -> p g s", s=8),
                    axis=mybir.AxisListType.X, op=Alu.min,
                )

                # fold to wrapped[0:16, 8g+a] = f0(t = 128g+16a+q)
                with nc.allow_non_contiguous_dma("tiny f0 index shuffle"):
                    for a in range(8):
                        nc.scalar.dma_start(
                            wv[0:16, 4 * j:4 * (j + 1), a:a + 1],
                            f0h[16 * a:16 * (a + 1), :],
                        )
                # replicate to all 16-partition groups (doubling)
                nc.scalar.dma_start(wrapped[16:32, sl], wrapped[0:16, sl])
                nc.scalar.dma_start(wrapped[32:64, sl], wrapped[0:32, sl])
                nc.scalar.dma_start(wrapped[64:128, sl], wrapped[0:64, sl])

                # transpose-mode gather: gg[p, c, t'] = LUT[f0(t)-1, 128c+p]
                gg = gg_pool.tile([128, 9 * NI], dt.bfloat16, name=f"gg{b}_{j}", tag="gg")
                nc.gpsimd.dma_gather(
                    gg[:].rearrange("p (c e) -> p c e", e=NI),
                    lut_d[:],
                    wrapped[:, sl],
                    num_idxs=NI,
                    num_idxs_reg=NI,
                    elem_size=LUT_W,
                    transpose=True,
                    queue_num=(2 * b + j) % NQ,
                )
                # one DMA for all 8 f-chunks: out[b, 128c+p, 512j+t'] = gg[p, c, t']
                nc.sync.dma_start(
                    out_d[b, :1024, NI * j:NI * (j + 1)].rearrange(
                        "(c p) t -> p c t", p=128
                    ),
                    gg[:, :8 * NI].rearrange("p (c t) -> p c t", t=NI),
                )
                # k=1024 row: partition 0 of chunk 8
                nc.sync.dma_start(
                    out_d[b, 1024:1025, NI * j:NI * (j + 1)],
                    gg[0:1, 8 * NI:9 * NI],
                )

    nc.compile()
    _CACHE["nc"] = nc
    return nc


def kernel(x: np.ndarray) -> np.ndarray:
    x = np.asarray(x)
    assert x.shape == (B, 1, F, T), x.shape
    nc = _build_nc()
    lut = _build_lut()
    ident = np.eye(128, dtype=np.float32)
    in_maps = [
        {
            "x": np.ascontiguousarray(x[NB * c:NB * (c + 1), 0]),
            "lut": lut,
            "ident": ident,
        }
        for c in range(NCORES)
    ]
    res = run_bass_kernel_spmd(nc, in_maps, core_ids=list(range(NCORES)))
    out = np.concatenate([res.results[c]["out"] for c in range(NCORES)], axis=0)
    return out[:, None, :, :].astype(np.float32)


# revision 15
# speedup vs baseline: 1.1733x; 1.1733x over previous
"""Trainium2 Bass kernel for nn_HarmonicEstimation (topk_masking).

Problem: x [16,1,1025,1024] f32 -> mask [16,1,1025,1024].
Per (batch, t) column over f-bins 1..1024: find top-5 peaks, f0 = min index
among peaks with value > 0.1 (else 0); output column = harmonic-comb mask
that depends ONLY on f0.

Strategy (8 cores, 2 batches/core, no communication):
  - Mask column is a pure function of f0 -> host-precomputed LUT, stored
    bf16 (tolerance 2e-2 >> bf16 error) with rows padded to 1152 elements
    (2304B, 256B-multiple as dma_gather requires). LUT rows are indexed by
    i = f0-1 (i in 0..1023); the invalid-column sentinel i=1023 coincides
    with f0=1024 whose row is all-0.5, exactly the wanted fallback, so the
    raw find_index8 output is directly the gather index.
  - Input loads with ONE DMA per batch (3-dim AP) into a [128, 8*1024]
    natural-layout tile; PE transposes 128x128 blocks into 2-bank
    [128,1024] PSUM tiles.
  - DVE max8/find_index8 read the full column from PSUM in one pass each
    (both have ~0.5us fixed cost - fewer, wider ops win). All small f0
    arithmetic runs on the Pool engine to keep DVE at exactly 2 ops/tile.
    f0 derives from a threshold rule exactly equivalent to top-5
    min-index: f0-1 = min{i : x[i+1] >= max(v5, nextafter(0.1))}.
  - The f0->gather index chain (fold to 16 partitions, replicate to 128
    as the SWDGE ucode requires, descriptor gen) is run per HALF-batch so
    all but the final half's chain hides under phase-1 compute.
  - One dma_gather per (batch, t-half) in TRANSPOSE mode (16-bit): the
    DMA engines gather LUT rows AND transpose them, so dst[p, c, t] =
    LUT[f0(t)-1, 128c+p] - natural [f-part, t-free] bf16 output tiles.
    (num_idxs=1024 in one gather fails on HW; 512 is safe.)
  - Output: ONE DMA per (batch, half) writes all 8 f-chunks (3-dim AP,
    1KB descriptors) + a tiny DMA for the k=1024 row; bf16, upcast on
    host.
"""

import os
import sys

for _p in ("/opt/trn_rl_repo", "/root/.axon_site/_ro/trn_rl_repo"):
    if os.path.isdir(_p) and _p not in sys.path:
        sys.path.insert(0, _p)

import numpy as np
import ml_dtypes

import concourse.bacc as bacc
import concourse.mybir as mybir
from concourse.bass_utils import run_bass_kernel_spmd
from concourse.tile import TileContext

dt = mybir.dt
Alu = mybir.AluOpType

B = 16          # full batch
NB = 2          # batches per core
NCORES = 8
F = 1025        # freq bins (0..1024)
T = 1024        # time columns
FT = 8          # f tiles of 128 covering bins 1..1024
TT = 8          # t tiles of 128
LUT_W = 1152    # LUT row padded to 1152 bf16 = 2304 B (multiple of 256)
NI = 512        # idxs per gather (1024 exceeds a HW ucode limit)
NQ = 4          # SWDGE queues (ucode max)
MAX_POWER = 0.1
# val > 0.1f  <=>  val >= nextafter(0.1f)
MP_NEXT = float(np.nextafter(np.float32(MAX_POWER), np.float32(1)))

_CACHE = {}


def _build_lut() -> np.ndarray:
    """LUT[i, k] = reference mask value at bin k given fundamental f0=i+1,
    bf16, rows padded to LUT_W. Mirrors reference.py arithmetic in f32.
    Rows i >= 1019 (f0 > 1020) are all-0.5; i=1023 doubles as the
    invalid-column sentinel."""
    if "lut" in _CACHE:
        return _CACHE["lut"]
    k = np.arange(F, dtype=np.int64)[None, :]           # [1, k]
    f0 = np.arange(1, 1025, dtype=np.int64)[:, None]    # [i, 1], f0 = i+1
    limit = F - 3 - 2  # F - FREQ_MARGIN - 2 = 1020
    m_mult = np.minimum((k + 3) // f0, limit // f0)
    i_last = m_mult * f0
    dist = np.abs(k - i_last).astype(np.float32)
    val = np.maximum(
        np.float32(1.0) - (np.float32(0.5) * dist) / np.float32(3.0),
        np.float32(0.5),
    )
    ok = (i_last >= f0) & (i_last >= k - 3)
    lut = np.where(ok, val, np.float32(0.5)).astype(np.float32)  # [1024, F]
    out = np.full((1024, LUT_W), 0.5, dtype=ml_dtypes.bfloat16)
    out[:, :F] = lut.astype(ml_dtypes.bfloat16)
    _CACHE["lut"] = out
    return out


def _build_nc():
    if "nc" in _CACHE:
        return _CACHE["nc"]
    from contextlib import ExitStack

    nc = bacc.Bacc(
        "TRN2", target_bir_lowering=False, debug=False, num_swdge_queues=NQ
    )
    x_in = nc.dram_tensor("x", [NB, F, T], dt.float32, kind="ExternalInput").ap()
    lut_d = nc.dram_tensor("lut", [1024, LUT_W], dt.bfloat16, kind="ExternalInput").ap()
    ident_d = nc.dram_tensor("ident", [128, 128], dt.float32, kind="ExternalInput").ap()
    out_d = nc.dram_tensor("out", [NB, F, T], dt.bfloat16, kind="ExternalOutput").ap()

    with TileContext(nc) as tc, ExitStack() as ctx:
        const_pool = ctx.enter_context(tc.tile_pool(name="constp", bufs=1))
        nat_pool = ctx.enter_context(tc.tile_pool(name="natp", bufs=16))
        gg_pool = ctx.enter_context(tc.tile_pool(name="ggp", bufs=4))
        psum_pool = ctx.enter_context(tc.tile_pool(name="psump", bufs=3, space="PSUM"))
        small_pool = ctx.enter_context(tc.tile_pool(name="smallp", bufs=2))

        ident_sb = const_pool.tile([128, 128], dt.float32, name="ident_sb")
        nc.sync.dma_start(ident_sb[:], ident_d[:])

        # ---- stage A: all input loads up-front (sync stream head, no
        # waits; one DMA per f-chunk so transposes of chunk h start as
        # soon as nat[h] lands) ----
        all_nats = []
        for b in range(NB):
            nats = []
            for h in range(FT):
                nat = nat_pool.tile([128, T], dt.float32, name=f"nat{b}_{h}", tag="nat")
                nc.sync.dma_start(
                    nat[:], x_in[b, 1 + h * 128: 1 + (h + 1) * 128, :]
                )
                nats.append(nat)
            all_nats.append(nats)

        deferred_outs = []
        for b in range(NB):
            nats = all_nats[b]
            # ---- per t-tile: transpose to PSUM, top-8 + thr/mask (DVE) ----
            vals = small_pool.tile([128, 8 * TT], dt.float32, name=f"vals{b}", tag="vals")
            idxs = small_pool.tile([128, 8 * TT], dt.uint16, name=f"idxs{b}", tag="idxs")
            mask = small_pool.tile([128, 8 * TT], dt.uint16, name=f"mask{b}", tag="mask")
            thr = small_pool.tile([128, TT], dt.float32, name=f"thr{b}", tag="thr")
            for g in range(TT):
                ps = psum_pool.tile([128, 1024], dt.float32, name=f"ps{b}_{g}", tag="ps")
                for h in range(FT):
                    nc.tensor.transpose(
                        ps[:, h * 128:(h + 1) * 128],
                        nats[h][:, g * 128:(g + 1) * 128],
                        ident_sb[:],
                    )
                vsl = vals[:, 8 * g:8 * g + 8]
                nc.vector.max(vsl, ps[:])
                nc.vector.max_index(idxs[:, 8 * g:8 * g + 8], vsl, ps[:])
                nc.vector.tensor_scalar(
                    thr[:, g:g + 1], vals[:, 8 * g + 4:8 * g + 5],
                    MP_NEXT, None, Alu.max,
                )
                nc.vector.tensor_scalar(
                    mask[:, 8 * g:8 * g + 8], vsl,
                    thr[:, g:g + 1], None, Alu.is_ge,
                )

            # ---- per t-half: f0-1, fold, replicate, gather, write out ----
            wrapped = small_pool.tile([128, 8 * TT], dt.int16, name=f"wrap{b}", tag="wrap")
            wv = wrapped[:].rearrange("p (g a) -> p g a", a=8)
            for j in range(2):
                sl = slice(32 * j, 32 * (j + 1))        # slots of this half
                # cand = mask ? idx : 1023 ; f0h = min over slots
                cand = small_pool.tile([128, 32], dt.uint16, name=f"cand{b}{j}", tag="cand")
                f0h = small_pool.tile([128, 4], dt.int16, name=f"f0h{b}{j}", tag="f0h")
                nc.vector.memset(cand[:], 1023)
                nc.vector.copy_predicated(cand[:], mask[:, sl], idxs[:, sl])
                nc.vector.tensor_reduce(
                    f0h[:], cand[:].rearrange("p (g s) -> p g s", s=8),
                    axis=mybir.AxisListType.X, op=Alu.min,
                )

                # fold to wrapped[0:16, 8g+a] = f0(t = 128g+16a+q).
                # Halves alternate between the sync and scalar engines so
                # the four chains' dispatch costs do not serialize; outs sit
                # at the tail of sync's stream where their waits block
                # nothing.
                eng = nc.sync if j == 0 else nc.scalar
                with nc.allow_non_contiguous_dma("tiny f0 index shuffle"):
                    for a in range(8):
                        eng.dma_start(
                            wv[0:16, 4 * j:4 * (j + 1), a:a + 1],
                            f0h[16 * a:16 * (a + 1), :],
                        )
                # replicate to all 16-partition groups (doubling)
                eng.dma_start(wrapped[16:32, sl], wrapped[0:16, sl])
                eng.dma_start(wrapped[32:64, sl], wrapped[0:32, sl])
                eng.dma_start(wrapped[64:128, sl], wrapped[0:64, sl])

                # transpose-mode gather: gg[p, c, t'] = LUT[f0(t)-1, 128c+p]
                gg = gg_pool.tile([128, 9 * NI], dt.bfloat16, name=f"gg{b}_{j}", tag="gg")
                nc.gpsimd.dma_gather(
                    gg[:].rearrange("p (c e) -> p c e", e=NI),
                    lut_d[:],
                    wrapped[:, sl],
                    num_idxs=NI,
                    num_idxs_reg=NI,
                    elem_size=LUT_W,
                    transpose=True,
                    queue_num=(2 * b + j) % NQ,
                )
                deferred_outs.append((b, j, gg))

        # ---- stage D: all output writes at the tail of sync's stream,
        # where their gather-completion waits cannot block anything ----
        for b, j, gg in deferred_outs:
            # one DMA for all 8 f-chunks: out[b, 128c+p, 512j+t'] = gg[p, c, t']
            nc.sync.dma_start(
                out_d[b, :1024, NI * j:NI * (j + 1)].rearrange(
                    "(c p) t -> p c t", p=128
                ),
                gg[:, :8 * NI].rearrange("p (c t) -> p c t", t=NI),
            )
            # k=1024 row: partition 0 of chunk 8
            nc.sync.dma_start(
                out_d[b, 1024:1025, NI * j:NI * (j + 1)],
                gg[0:1, 8 * NI:9 * NI],
            )

    nc.compile()
    _CACHE["nc"] = nc
    return nc


def kernel(x: np.ndarray) -> np.ndarray:
    x = np.asarray(x)
    assert x.shape == (B, 1, F, T), x.shape
    nc = _build_nc()
    lut = _build_lut()
    ident = np.eye(128, dtype=np.float32)
    in_maps = [
        {
            "x": np.ascontiguousarray(x[NB * c:NB * (c + 1), 0]),
            "lut": lut,
            "ident": ident,
        }
        for c in range(NCORES)
    ]
    res = run_bass_kernel_spmd(nc, in_maps, core_ids=list(range(NCORES)))
    out = np.concatenate([res.results[c]["out"] for c in range(NCORES)], axis=0)
    return out[:, None, :, :].astype(np.float32)
